# revision 1
# baseline (speedup 1.0000x reference)
"""Trainium2 Bass kernel for the Boat Dynamic System problem.

Math: out[b, c] = sum_f V[b, f] * coeffs[c, f] where V = [base, pro*base,
rud*base] and base = 15 quadratic monomials of s = (u, v, r, Pf).

Folding pro/rud (scalars picked from cmd on the host) gives an effective
[4, 15] coefficient matrix, i.e. out_c = s~^T Qc s~ with s~ = (1, u, v, r, Pf).
Any quadratic form is an exact linear combination of squares of 15 fixed
linear functionals w_j (e_a and e_a + e_b patterns over the 5-dim s~):

    out_c(b) = sum_j lam[c, j] * (w_j . s~(b))^2

Device pipeline per [128, 512] tile (16384 batch elements, per core):
  1. contiguous DMA of state (natural layout)
  2. PE transposes of four [128, 128] blocks -> partition q = 4n+f
     (n = batch-sub 0..31, f = component), column = batch-chunk; fp32r
     (1.5 cyc/col) — its rounding is absorbed by M1's own fp32r truncation
  3. evac PSUM -> SBUF (cast to fp32r when enabled)
  4. M1: row-tiled K=32 matmuls (row group 32t) project onto the 15 w_j,
     Y split into two independent 2-bank halves for fine PSUM recycling
  5. ACT Square(Y + bias) per half, PSUM -> SBUF (bias = w_j constant)
  6. M2: accumulating M=64 matmul pairs (lhsT = [lam|0] then [0|lam]) at
     dst partition 0 -> [64, 1024] PSUM (fp32r rejects dst>0 for M>32)
  7. DVE evac, PE transposes [64,128] blocks back to batch-natural
     [bit-exact], ACT evac, contiguous DMA out

Matmul dtype is fp32r (full-rate; tf32-like rounding) or fp32 (quarter-rate,
exact) via BOAT_PRECISE=1.
"""

import os

import numpy as np

NCORES = 8
B = 2097152
BS = B // NCORES          # 262144 rows per core
DT = 0.01
NTILES = 16               # tiles per core
TILE_B = BS // NTILES     # 16384 batch elements per tile
NCOL = 512                # columns per tile (32 batch elements per column)

_PAIRS = [(a, b) for a in range(5) for b in range(a, 5)]  # 15 (a<=b) pairs
_MONO2FEAT = {
    (0, 0): 0, (0, 1): 1, (0, 2): 2, (0, 3): 3, (0, 4): 4,
    (1, 1): 5, (1, 2): 6, (1, 3): 7, (1, 4): 8,
    (2, 2): 9, (2, 3): 10, (2, 4): 11,
    (3, 3): 12, (3, 4): 13,
    (4, 4): 14,
}

_NC_CACHE = {}
LAST_RESULT = [None]


def _square_basis():
    """15 fixed vectors w_j in R^5 whose squared functionals span quadratics."""
    W5 = np.zeros((15, 5), dtype=np.float64)
    for j, (a, b) in enumerate(_PAIRS):
        W5[j, a] += 1.0
        if b != a:
            W5[j, b] += 1.0
    # M[m, j] = coefficient of monomial m=(x,y) in (w_j . s~)^2
    M = np.zeros((15, 15), dtype=np.float64)
    for m, (x, y) in enumerate(_PAIRS):
        for j in range(15):
            M[m, j] = W5[j, x] * W5[j, y] * (1.0 if x == y else 2.0)
    return W5, M


def _host_weights(t, cmd, coeffs):
    """Fold cmd/coeffs into the device weight tensors (all tiny)."""
    idx = int(np.round(float(np.asarray(t).reshape(-1)[0]) / DT))
    pro = float(cmd[idx, 0])
    rud = float(cmd[idx, 1])
    cf = np.asarray(coeffs, dtype=np.float64)
    ceff = cf[:, 0:15] + pro * cf[:, 15:30] + rud * cf[:, 30:45]  # [4, 15]

    gamma = np.zeros((4, 15), dtype=np.float64)
    for m, (x, y) in enumerate(_PAIRS):
        gamma[:, m] = ceff[:, _MONO2FEAT[(x, y)]]

    W5, M = _square_basis()
    lam45 = np.linalg.solve(M, gamma.T).T  # [4, 15]

    # wexp4 [128, 120]: wexp4[32t + 4g + f, g*15 + j] = W5[j, 1+f]
    # (same [32, 120] block replicated at each row-group t for row tiling)
    wexp4 = np.zeros((128, 120), dtype=np.float32)
    for t_ in range(4):
        for g in range(8):
            for j in range(15):
                for f in range(4):
                    wexp4[32 * t_ + 4 * g + f, g * 15 + j] = W5[j, 1 + f]

    biasw = np.zeros((120, 1), dtype=np.float32)
    for g in range(8):
        for j in range(15):
            biasw[g * 15 + j, 0] = W5[j, 0]

    lam = np.zeros((120, 32), dtype=np.float32)
    for g in range(8):
        for j in range(15):
            for c in range(4):
                lam[g * 15 + j, 4 * g + c] = lam45[c, j]

    # fp32r col-tiled matmuls reject dst partition 96, so M2 runs as
    # accumulating M=64 pairs: lamA = [lam | 0], lamB = [0 | lam]
    lamAB = np.zeros((120, 128), dtype=np.float32)
    lamAB[:, 0:32] = lam        # lamA = lamAB[:, 0:64]
    lamAB[:, 96:128] = lam      # lamB = lamAB[:, 64:128]

    return wexp4, biasw, lamAB


def _build_nc(precise: bool):
    import concourse.bacc as bacc
    import concourse.mybir as mybir
    import concourse.tile as tile
    from concourse.masks import make_identity

    nc = bacc.Bacc("TRN2", target_bir_lowering=False, debug=False)
    f32 = mybir.dt.float32
    mmdt = f32 if precise else mybir.dt.float32r
    Square = mybir.ActivationFunctionType.Square

    state = nc.dram_tensor("state", [BS, 4], mmdt, kind="ExternalInput")
    wexp_d = nc.dram_tensor("wexp", [128, 120], mmdt, kind="ExternalInput")
    biasw_d = nc.dram_tensor("biasw", [120, 1], f32, kind="ExternalInput")
    lam_d = nc.dram_tensor("lam", [120, 128], mmdt, kind="ExternalInput")
    out = nc.dram_tensor("out", [BS, 4], f32, kind="ExternalOutput")

    state_r = state[:, :].rearrange(
        "(T blk p n) f -> T p blk n f", T=NTILES, blk=4, p=128, n=32
    )
    out_r = out[:, :].rearrange(
        "(T blk p n) f -> T p blk n f", T=NTILES, blk=4, p=128, n=32
    )

    with tile.TileContext(nc) as tc:
        with (
            tc.tile_pool(name="consts", bufs=1) as cpool,
            tc.tile_pool(name="sb", bufs=4) as sb,
            tc.tile_pool(name="ps", bufs=1, space="PSUM") as ps,
        ):
            ident = cpool.tile([128, 128], f32)
            make_identity(nc, ident[:])
            identr = cpool.tile([128, 128], mmdt)
            nc.vector.tensor_copy(out=identr[:], in_=ident[:])
            wexp_sb = cpool.tile([128, 120], mmdt)
            nc.sync.dma_start(out=wexp_sb[:], in_=wexp_d[:, :])
            biasw_sb = cpool.tile([120, 1], f32)
            nc.sync.dma_start(out=biasw_sb[:], in_=biasw_d[:, :])
            lam_sb = cpool.tile([120, 128], mmdt)
            nc.sync.dma_start(out=lam_sb[:], in_=lam_d[:, :])

            for T in range(NTILES):
                xn = sb.tile([128, NCOL], mmdt)
                nc.sync.dma_start(out=xn[:], in_=state_r[T])

                spsum = ps.tile([128, NCOL], mmdt)
                for blk in range(4):
                    nc.tensor.transpose(
                        out=spsum[:, blk * 128:(blk + 1) * 128],
                        in_=xn[:, blk * 128:(blk + 1) * 128],
                        identity=identr[:],
                    )
                # evac (and round to fp32r when enabled) on DVE
                ssb = sb.tile([128, NCOL], mmdt)
                nc.vector.tensor_copy(out=ssb[:], in_=spsum[:])

                # M1: row-tiled K=32 matmuls; Y split into two independent
                # 2-bank halves so squares overlap M1 and free banks earlier
                fsb = sb.tile([120, 4 * NCOL], mmdt)
                for h in range(2):
                    yps = ps.tile([120, 2 * NCOL], f32, tag=f"y{h}")
                    for u in range(2):
                        t_ = 2 * h + u
                        nc.tensor.matmul(
                            out=yps[:, u * NCOL:(u + 1) * NCOL],
                            lhsT=wexp_sb[32 * t_:32 * (t_ + 1), :],
                            rhs=ssb[32 * t_:32 * (t_ + 1), :],
                            start=True,
                            stop=True,
                            tile_position=(32 * t_, 0),
                        )
                    nc.scalar.activation(
                        out=fsb[:, h * 2 * NCOL:(h + 1) * 2 * NCOL],
                        in_=yps[:],
                        func=Square,
                        bias=biasw_sb[:, 0:1],
                        scale=1.0,
                    )

                # M2: accumulating M=64 pairs, both at dst partition 0
                # (fp32r matmul requires dst 0 for M>32); the two t-halves
                # land side by side in the free dim: ops2[q2, half*512+col]
                # with q2 = 32*ab + 4g + c, t = 2*half + ab.
                ops2 = ps.tile([64, 2 * NCOL], f32)
                for ab in range(2):
                    for half in range(2):
                        t_ = 2 * half + ab
                        nc.tensor.matmul(
                            out=ops2[0:64, half * NCOL:(half + 1) * NCOL],
                            lhsT=lam_sb[:, 64 * ab:64 * (ab + 1)],
                            rhs=fsb[:, t_ * NCOL:(t_ + 1) * NCOL],
                            start=(ab == 0),
                            stop=(ab == 1),
                            tile_position=(0, 0),
                            skip_group_check=True,
                        )
                # evac halves to partition strips 0-63 / 64-127 of one tile
                osb = sb.tile([128, NCOL], f32)
                nc.vector.tensor_copy(out=osb[0:64, :], in_=ops2[0:64, 0:NCOL])
                nc.vector.tensor_copy(
                    out=osb[64:128, :], in_=ops2[0:64, NCOL:2 * NCOL]
                )

                # transpose back: 4 full [128, 128] blocks; free layout
                # blk*128 + 64*half + q2 = blk*128 + 32t + 4g + c
                tps = ps.tile([128, NCOL], f32)
                for blk in range(4):
                    nc.tensor.transpose(
                        out=tps[:, blk * 128:(blk + 1) * 128],
                        in_=osb[:, blk * 128:(blk + 1) * 128],
                        identity=ident[:],
                    )
                oub = sb.tile([128, NCOL], f32)
                nc.vector.tensor_copy(out=oub[:], in_=tps[:])
                nc.sync.dma_start(out=out_r[T], in_=oub[:])

    nc.finalize()
    return nc


def _ensure_ntff_hook():
    """Install the axon NTFF profiling hook if the image's antenv lacks it."""
    import sys
    import types
    try:
        from antenv.axon_hooks import get_axon_ntff_profile_hook  # noqa: F401
        return
    except ImportError:
        pass
    try:
        import antenv
        from trn_agent_boot.trn_boot import _ntff_profile_via_ctypes
        mod = types.ModuleType("antenv.axon_hooks")
        store = [None]
        mod.set_axon_ntff_profile_hook = lambda h: store.__setitem__(0, h)
        mod.get_axon_ntff_profile_hook = lambda: store[0]
        sys.modules["antenv.axon_hooks"] = mod
        antenv.axon_hooks = mod
        mod.set_axon_ntff_profile_hook(
            _ntff_profile_via_ctypes("/opt/axon/libaxon_pjrt.so")
        )
        import concourse.bass_utils as bu
        bu.upload_artifacts = lambda tmpdir: tmpdir
    except Exception as e:  # profiling is best-effort
        print(f"ntff hook install failed: {e}")


def kernel(t, state, cmd, coeffs):
    from concourse.bass_utils import run_bass_kernel_spmd

    trace = bool(int(os.environ.get("BOAT_TRACE", "0")))
    if trace:
        _ensure_ntff_hook()

    t = np.asarray(t)
    state = np.ascontiguousarray(np.asarray(state, dtype=np.float32))
    cmd = np.asarray(cmd, dtype=np.float32)
    coeffs = np.asarray(coeffs, dtype=np.float32)

    wexp4, biasw, lam = _host_weights(t, cmd, coeffs)

    precise = bool(int(os.environ.get("BOAT_PRECISE", "0")))
    key = ("nc", precise)
    if key not in _NC_CACHE:
        _NC_CACHE[key] = _build_nc(precise)
    nc = _NC_CACHE[key]

    in_maps = []
    for k in range(NCORES):
        shard = np.ascontiguousarray(state[k * BS:(k + 1) * BS])
        in_maps.append(
            {"state": shard, "wexp": wexp4, "biasw": biasw, "lam": lam}
        )

    res = run_bass_kernel_spmd(
        nc,
        in_maps,
        core_ids=list(range(NCORES)),
        trace=trace,
    )
    LAST_RESULT[0] = res
    return np.concatenate([r["out"] for r in res.results], axis=0)

